# revision 24
# baseline (speedup 1.0000x reference)
"""Trainium2 Bass kernel for nn_IntraCycleMoELayer (MoE routing, 8 cores).

Strategy
--------
The reference computes all E=8 experts densely, but the top-2 gate zeroes all
but 2 experts per batch row.  Real work per row: 2 routed expert MLP blocks +
1 general MLP block of
  LN(gelu_tanh(x @ w1 + b1) @ w2 + b2 + x) * gamma + beta
over [L=512 tokens, D=768] with DFF=3072.  Routed jobs with a renormalized
gate below GATE_SKIP (1e-2) are dropped (~0.1% output error).

Precision plan (gate: rel_err < 2e-2; fp8-e4m3 DoubleRow gives 2x PE
throughput): routed experts run fully in fp8; the general expert (weight 1.0
in the output sum, the accuracy-critical term) runs with fp8 mm1 + fp16 mm2
on half the rows and fully fp16 on the rest.  Remaining routed work is
token-split so all 8 SPMD cores carry the same slot structure:
  [fp8 whole, fp8 whole, hybrid gen, fp8 half, fp16 gen]
fp8 accuracy tricks:
  * w1 is pre-scaled by 128 and w2 by 256 before e4m3 quantization so the
    weight magnitudes (std ~0.02-0.04) land in e4m3's normal range instead
    of its denormals (this alone is worth ~2x in error).  The w1 scale is
    undone for free by the gelu activation's `scale` operand; the w2 scale
    is never undone - the residual xr is pre-scaled by 256 on the host and
    LayerNorm is scale invariant.
  * gelu output h is quantized to fp8 (routed) / fp16 (hybrid) directly by
    the scalar engine.
  * gate*gamma folds into the LN rstd constant (c_mul staged per slot), so
    no per-chunk gamma/beta DVE passes are needed when gamma is constant
    and beta is zero; rstd = rsqrt(var*c_mul + c_add) is computed on the
    DVE with the int32 exponent-halving seed + one Newton step (no
    scalar-engine Sqrt => no activation-table thrash against gelu).

Engine/ring discipline (descriptor ISSUE costs ~650ns on the issuing
engine, and a waiting descriptor head-blocks its ring):
  * sync ring: slot-0 critical xt + w1 first, deferred non-critical early
    loads behind an early mm1 matmul, then y outputs (fp16).
  * gpsimd ring: only never-blocking early bulk (b1/lnc, first w2, xr).
  * every later WAR-gated weight/xr load issues from the scalar engine's
    instruction stream at its slot, where program order has already
    released the target buffer - so no ring ever stalls on a WAR wait.
"""
import numpy as np

import concourse.bass as bass
import concourse.mybir as mybir
import concourse.tile as tile
from concourse import bacc
from concourse.bass import ts
from concourse import bass_utils

B, L, D, DFF, DLLM, E, TOPK = 16, 512, 768, 3072, 4096, 8, 2
EPS_GATE = 1e-9
LN_EPS = 1e-5
NCORES = 8
GEN = E                               # weight-set id of the general expert
GATE_SKIP = 0.01                      # drop routed jobs with tiny gates
W1_SCALE = 128.0                      # fp8 weight pre-scales (powers of 2)
W2_SCALE = 256.0
KC1, MC1 = D // 128, DFF // 128       # 6, 24
KC2 = DFF // 128                      # 24
GPC = B // NCORES                     # general jobs per core = 2
dt = mybir.dt
F8NP = dt.np(dt.float8e4)
# the MLP nonlinearity; CoreSim lacks Gelu_apprx_tanh, so sim tests swap this
_ACT_FUNC = mybir.ActivationFunctionType.Gelu_apprx_tanh

_cache = {}  # spec -> finalized nc


def _router(cycle_numbers, DKP_embeddings, gate_We, gate_Wc, gate_b, gate_Wo,
            gate_bo):
    """Replicate the reference router in fp32 numpy: top-2 indices + gates."""
    h = np.maximum(
        DKP_embeddings @ gate_We + cycle_numbers @ gate_Wc + gate_b, 0.0)
    logits = h @ gate_Wo + gate_bo                       # [B, E]
    idx = np.argsort(-logits, axis=1, kind="stable")[:, :TOPK]
    m = logits.max(axis=1, keepdims=True)
    p = np.exp(logits - m)
    p /= p.sum(axis=1, keepdims=True)
    mask = np.zeros_like(p)
    mask[np.arange(logits.shape[0])[:, None], idx] = 1.0
    gated = p * mask
    gated = gated / (gated.sum(axis=1, keepdims=True) + EPS_GATE)
    return idx, gated


def _build_nc(spec):
    """Build the SPMD per-core program.

    spec = (n18, n28, n116, n216, fold_gb, slots); each slot is
      (tok, mode, lw18, lw28, lw116, lw216):
      tok  - token count for this slot (multiple of 128)
      mode - 8 (fp8 DoubleRow), 9 (hybrid: fp8 mm1 + fp16 mm2), 16 (fp16)
      lw** - weight-pool slot to DMA before this job (None = reuse previous)
    """
    if spec in _cache:
        return _cache[spec]
    n18, n28, n116, n216, fold_gb, slots = spec
    n_slots = len(slots)

    nc = bacc.Bacc("TRN2", target_bir_lowering=False, debug=False)
    w18_d = w28_d = w116_d = w216_d = None
    if n18:
        w18_d = nc.dram_tensor("w18", [n18, D, DFF], dt.float8e4, kind="ExternalInput")
    if n28:
        w28_d = nc.dram_tensor("w28", [n28, DFF, D], dt.float8e4, kind="ExternalInput")
    if n116:
        w116_d = nc.dram_tensor("w116", [n116, D, DFF], dt.float16, kind="ExternalInput")
    if n216:
        w216_d = nc.dram_tensor("w216", [n216, DFF, D], dt.float16, kind="ExternalInput")
    xt_d, xr_d, y_d = [], [], []
    for j, (tok, mode, _, _, _, _) in enumerate(slots):
        xdt = dt.float8e4 if mode in (8, 9) else dt.float16
        xt_d.append(nc.dram_tensor(f"xt{j}", [D, tok], xdt, kind="ExternalInput"))
        xr_d.append(nc.dram_tensor(f"xr{j}", [tok, D], dt.float16, kind="ExternalInput"))
        y_d.append(nc.dram_tensor(f"y{j}", [tok, D], dt.float16, kind="ExternalOutput"))
    b1_d = nc.dram_tensor("b1", [128, n_slots, MC1], dt.float32, kind="ExternalInput")
    gb_d = nc.dram_tensor("gb", [n_slots, 2, D], dt.float16, kind="ExternalInput")
    # per-slot LN constants: [:,:,0]=c_mul, [:,:,1]=c_add so that
    # rstd' = rsqrt(var*c_mul + c_add); with fold_gb, c_mul=1/(g*gamma)^2
    # and the gamma/beta DVE passes are skipped entirely.
    lnc_d = nc.dram_tensor("lnc", [128, n_slots, 2], dt.float32, kind="ExternalInput")

    gelu = _ACT_FUNC
    DR = mybir.MatmulPerfMode.DoubleRow
    nf8slots = sum(1 for s in slots if s[1] in (8, 9))

    with tile.TileContext(nc) as tc, \
         tc.tile_pool(name="w18p", bufs=1) as w18p, \
         tc.tile_pool(name="w28p", bufs=1) as w28p, \
         tc.tile_pool(name="w116p", bufs=1) as w116p, \
         tc.tile_pool(name="w216p", bufs=1) as w216p, \
         tc.tile_pool(name="xt8p", bufs=min(3, max(1, nf8slots))) as xt8p, \
         tc.tile_pool(name="xt16p", bufs=1) as xt16p, \
         tc.tile_pool(name="h8p", bufs=2) as h8p, \
         tc.tile_pool(name="h16p", bufs=1) as h16p, \
         tc.tile_pool(name="xrp", bufs=2) as xrp, \
         tc.tile_pool(name="gbp", bufs=2) as gbp, \
         tc.tile_pool(name="rp", bufs=3) as rp, \
         tc.tile_pool(name="sp", bufs=2) as sp, \
         tc.tile_pool(name="cp", bufs=1) as cp, \
         tc.tile_pool(name="php", bufs=4, space="PSUM") as php, \
         tc.tile_pool(name="pop", bufs=2, space="PSUM") as pop:

        from concourse.bass import _add_dep_helper

        b1_all = cp.tile([128, n_slots, MC1], dt.float32)
        lnc_all = cp.tile([128, n_slots, 2], dt.float32)

        # PE warmup: matmuls on zeros so the HAM clock-gate ramps while the
        # first weight DMAs are in flight (sized to end as slot0's w1 lands).
        warm_z = cp.tile([128, 512], dt.float16)
        nc.vector.memset(warm_z, 0.0)
        for _ in range(10):
            wp_t = php.tile([128, 512], dt.float32, tag="ph")
            nc.tensor.matmul(wp_t, lhsT=warm_z[:, 0:128], rhs=warm_z,
                             start=True, stop=True)

        first_mm = None      # anchor for delaying non-critical head DMAs
        deferred = []        # DMA insts to hook behind first_mm

        def dma(engine, dst, src, defer):
            d = engine.dma_start(dst, src)
            if defer:
                deferred.append(d)
            return d

        # ---- weight / xt loads, emitted with explicit queue ordering ----
        w18_sb = [None] * n_slots
        w28_sb = [None] * n_slots
        w116_sb = [None] * n_slots
        w216_sb = [None] * n_slots
        xt_sb = [None] * n_slots
        H = DFF // 2

        # per-k(-group) full-row descriptors: DMA descriptor ISSUE costs
        # ~650ns on the issuing engine, so fewer/bigger beats many small.
        def load_w18(j, u, eng, defer):
            t = w18p.tile([128, KC1, DFF], dt.float8e4, tag="w18")
            src = w18_d[u].rearrange("(ko p) n -> p ko n", p=128)
            for k in range(KC1):
                dma(eng, t[:, k, :], src[:, k, :], defer)
            w18_sb[j] = t

        def load_w28(j, u, eng, defer):
            t = w28p.tile([128, KC2, D], dt.float8e4, tag="w28")
            src = w28_d[u].rearrange("(ko p) n -> p ko n", p=128)
            for k in range(0, KC2, 6):
                dma(eng, t[:, k:k + 6, :], src[:, k:k + 6, :], defer)
            w28_sb[j] = t

        def load_w116(j, u, eng, defer):
            t = w116p.tile([128, KC1, DFF], dt.float16, tag="w116")
            src = w116_d[u].rearrange("(ko p) n -> p ko n", p=128)
            for k in range(0, KC1, 2):
                dma(eng, t[:, k:k + 2, :], src[:, k:k + 2, :], defer)
            w116_sb[j] = t

        def load_w216(j, u, eng, defer):
            t = w216p.tile([128, KC2, D], dt.float16, tag="w216")
            src = w216_d[u].rearrange("(ko p) n -> p ko n", p=128)
            for k in range(0, KC2, 6):
                dma(eng, t[:, k:k + 6, :], src[:, k:k + 6, :], defer)
            w216_sb[j] = t

        def load_xt(j, defer):
            tok, mode = slots[j][0], slots[j][1]
            if mode in (8, 9):
                t = xt8p.tile([128, KC1, tok], dt.float8e4, tag="xt8")
            else:
                t = xt16p.tile([128, KC1, tok], dt.float16, tag="xt16")
            src = xt_d[j].rearrange("(ko p) t -> p ko t", p=128)
            dma(nc.sync, t, src, defer)
            xt_sb[j] = t

        xr_sb = [None] * n_slots
        gb_sb = [None] * n_slots

        def load_xr_gb(j, eng, defer):
            tok = slots[j][0]
            t = xrp.tile([128, tok // 128, D], dt.float16, tag="xr")
            src = xr_d[j].rearrange("(t p) d -> p t d", p=128)
            dma(eng, t, src, defer)
            xr_sb[j] = t
            if not fold_gb:   # gamma/beta tiles only needed un-folded
                g = gbp.tile([128, 2, D], dt.float16, tag="gb")
                gap = gb_d[j]
                dma(eng, g,
                    bass.AP(tensor=gap.tensor, offset=gap.offset,
                            ap=[[0, 128], *gap.ap]), defer)
                gb_sb[j] = g

        # Ring discipline:
        #   sync:   xt0, w18[0] | deferred: other xt, first w116 | y outs
        #   gpsimd: b1/lnc | deferred: first w28, xr's, first w216
        #   scalar stream (at the owning slot, in program order): every
        #     later (WAR-gated) weight load - its program position already
        #     satisfies the WAR, so no ring is ever head-blocked by a wait.
        f8_js = [j for j in range(n_slots) if slots[j][1] in (8, 9)]
        gen_js = [j for j in range(n_slots) if slots[j][1] == 16]
        j0 = 0
        load_xt(j0, defer=False)
        tok0, mode0, w18l0, w28l0, w116l0, w216l0 = slots[j0]
        if w18l0 is not None:
            load_w18(j0, w18l0, nc.sync, defer=False)
        if w116l0 is not None:
            load_w116(j0, w116l0, nc.sync, defer=False)
        dma(nc.gpsimd, b1_all, b1_d[:], False)
        dma(nc.gpsimd, lnc_all, lnc_d[:], False)
        for j in range(1, n_slots):
            load_xt(j, defer=True)
        if w28l0 is not None:
            load_w28(j0, w28l0, nc.gpsimd, defer=True)
        if w216l0 is not None:
            load_w216(j0, w216l0, nc.gpsimd, defer=True)
        for j in f8_js[:2]:   # first two fit xrp's bufs without WAR waits
            load_xr_gb(j, nc.gpsimd, defer=True)
        # first-use (non-WAR) pool loads of later slots: fine on the rings
        done_first = {"w116": w116l0 is not None, "w216": w216l0 is not None}
        for j in range(1, n_slots):
            if slots[j][4] is not None and not done_first["w116"]:
                load_w116(j, slots[j][4], nc.sync, defer=True)
                done_first["w116"] = True
            if slots[j][5] is not None and not done_first["w216"]:
                load_w216(j, slots[j][5], nc.gpsimd, defer=True)
                done_first["w216"] = True

        # ---- compute ----
        last = {"w18": w18_sb[j0], "w28": w28_sb[j0],
                "w116": w116_sb[j0], "w216": w216_sb[j0]}
        for j, (tok, mode, lw18, lw28, lw116, lw216) in enumerate(slots):
            TCj = tok // 128
            if xr_sb[j] is None:
                # WAR-gated xr loads issue from the scalar stream here;
                # program order has already released the buffer.
                load_xr_gb(j, nc.scalar, defer=False)
            # later (WAR-gated) weight loads: issue from the scalar stream
            # right here; program order has already released the buffers.
            if j != j0:
                if lw18 is not None and w18_sb[j] is None:
                    load_w18(j, lw18, nc.scalar, defer=False)
                if lw28 is not None and w28_sb[j] is None:
                    load_w28(j, lw28, nc.scalar, defer=False)
                if lw116 is not None and w116_sb[j] is None:
                    load_w116(j, lw116, nc.scalar, defer=False)
                if lw216 is not None and w216_sb[j] is None:
                    load_w216(j, lw216, nc.scalar, defer=False)
            for pool, sb in (("w18", w18_sb), ("w28", w28_sb),
                             ("w116", w116_sb), ("w216", w216_sb)):
                if sb[j] is not None:
                    last[pool] = sb[j]
                else:
                    sb[j] = last[pool]

            # mm1 + gelu: h^T [DFF on partitions, tokens free]
            if mode == 8:
                h_sb = h8p.tile([128, KC2, tok], dt.float8e4, tag="h8")
            else:
                h_sb = h16p.tile([128, KC2, tok], dt.float16, tag="h16")
            for m in range(MC1):
                ph = php.tile([128, 512], dt.float32, tag="ph")
                if mode in (8, 9):
                    for kp in range(KC1 // 2):
                        mm = nc.tensor.matmul(
                            ph[:, 0:tok],
                            lhsT=w18_sb[j][:, 2 * kp:2 * kp + 2, ts(m, 128)],
                            rhs=xt_sb[j][:, 2 * kp:2 * kp + 2, :],
                            start=(kp == 0), stop=(kp == KC1 // 2 - 1),
                            perf_mode=DR)
                        if first_mm is None and j == 0 and m == 12 and kp == 0:
                            first_mm = mm
                            for d in deferred:
                                _add_dep_helper(
                                    d.ins, first_mm.ins, sync=True,
                                    reason="delay non-critical head DMA")
                else:
                    for k in range(KC1):
                        mm = nc.tensor.matmul(
                            ph[:, 0:tok],
                            lhsT=w116_sb[j][:, k, ts(m, 128)],
                            rhs=xt_sb[j][:, k, :],
                            start=(k == 0), stop=(k == KC1 - 1))
                        if first_mm is None and j == 0 and m == 12 and k == 0:
                            first_mm = mm
                            for d in deferred:
                                _add_dep_helper(
                                    d.ins, first_mm.ins, sync=True,
                                    reason="delay non-critical head DMA")
                nc.scalar.activation(
                    out=h_sb[:, m, :], in_=ph[:, 0:tok], func=gelu,
                    bias=b1_all[:, j, m:m + 1],
                    scale=(1.0 / W1_SCALE) if mode in (8, 9) else 1.0)

            # mm2 + residual + LN per 128-token chunk; rstd comes from a
            # DVE-only Newton rsqrt (no scalar-engine act-table involvement),
            # so each chunk completes independently right behind its matmuls.
            for t in range(TCj):
                po = pop.tile([128, D], dt.float32, tag="po")
                if mode == 8:
                    for kp in range(KC2 // 2):
                        lh = h_sb[:, 2 * kp:2 * kp + 2, ts(t, 128)]
                        nc.tensor.matmul(po[:, 0:512], lhsT=lh,
                                         rhs=w28_sb[j][:, 2 * kp:2 * kp + 2, 0:512],
                                         start=(kp == 0), stop=(kp == KC2 // 2 - 1),
                                         perf_mode=DR)
                        nc.tensor.matmul(po[:, 512:D], lhsT=lh,
                                         rhs=w28_sb[j][:, 2 * kp:2 * kp + 2, 512:D],
                                         start=(kp == 0), stop=(kp == KC2 // 2 - 1),
                                         perf_mode=DR)
                else:
                    for k in range(KC2):
                        lh = h_sb[:, k, ts(t, 128)]
                        nc.tensor.matmul(po[:, 0:512], lhsT=lh,
                                         rhs=w216_sb[j][:, k, 0:512],
                                         start=(k == 0), stop=(k == KC2 - 1))
                        nc.tensor.matmul(po[:, 512:D], lhsT=lh,
                                         rhs=w216_sb[j][:, k, 512:D],
                                         start=(k == 0), stop=(k == KC2 - 1))
                r_sb = rp.tile([128, D], dt.float32, tag="r")
                nc.vector.tensor_add(r_sb, po, xr_sb[j][:, t, :])
                stats = sp.tile([128, 3, 6], dt.float32, tag="st")
                for s in range(3):
                    nc.vector.bn_stats(stats[:, s, :], r_sb[:, ts(s, 256)])
                mv = sp.tile([128, 2], dt.float32, tag="mv")
                nc.vector.bn_aggr(mv, stats)
                # rstd' = rsqrt(var*c_mul + c_add), one Newton iteration
                # seeded by the int32 exponent-halving trick, on the DVE.
                v = sp.tile([128, 1], dt.float32, tag="v")
                nc.vector.tensor_scalar(out=v, in0=mv[:, 1:2],
                                        scalar1=lnc_all[:, j, 0:1],
                                        scalar2=lnc_all[:, j, 1:2],
                                        op0=mybir.AluOpType.mult,
                                        op1=mybir.AluOpType.add)
                yb = sp.tile([128, 1], dt.int32, tag="yb")
                nc.vector.tensor_scalar(out=yb, in0=v.bitcast(dt.int32),
                                        scalar1=1, scalar2=None,
                                        op0=mybir.AluOpType.logical_shift_right)
                nc.vector.tensor_scalar(out=yb, in0=yb,
                                        scalar1=-1, scalar2=0x5f3759df,
                                        op0=mybir.AluOpType.mult,
                                        op1=mybir.AluOpType.add)
                yf = yb.bitcast(dt.float32)
                nt = sp.tile([128, 1], dt.float32, tag="nt")
                nc.vector.tensor_mul(nt, yf, yf)
                nc.vector.tensor_mul(nt, nt, v)
                nc.gpsimd.tensor_scalar(out=nt, in0=nt, scalar1=-0.5,
                                        scalar2=1.5,
                                        op0=mybir.AluOpType.mult,
                                        op1=mybir.AluOpType.add)
                nc.vector.tensor_mul(yf, yf, nt)
                r16 = rp.tile([128, D], dt.float16, tag="r16")
                nc.gpsimd.tensor_scalar(out=(r16 if fold_gb else r_sb),
                                        in0=r_sb,
                                        scalar1=mv[:, 0:1],
                                        scalar2=yf,
                                        op0=mybir.AluOpType.subtract,
                                        op1=mybir.AluOpType.mult)
                if not fold_gb:
                    nc.vector.tensor_mul(r_sb, r_sb, gb_sb[j][:, 0, :])
                    nc.vector.tensor_add(r16, r_sb, gb_sb[j][:, 1, :])
                nc.sync.dma_start(
                    y_d[j].rearrange("(t p) d -> p t d", p=128)[:, t, :], r16)

    nc.finalize()
    _cache[spec] = nc
    return nc


def _schedule(idx, gated, fold_gb):
    """Build the SPMD slot structure + per-core assignment.

    Returns (spec, assign) where assign[c][j] =
      (row, set_id, gate, tok_off, tok_cnt, real)
    """
    rjobs = []
    for r in range(B):
        for e in idx[r]:
            g = float(gated[r, e])
            if g >= GATE_SKIP:
                rjobs.append((r, int(e), g))
    # keep the largest-gate jobs whole; split the smallest-gate ones
    rjobs.sort(key=lambda t: -t[2])
    R = len(rjobs)
    hpc = -(-2 * R // NCORES)            # routed half-slots per core
    nf, nh = hpc // 2, hpc % 2
    whole_cap, half_cap = NCORES * nf, NCORES * nh
    n_split = max(0, R - whole_cap)
    whole = [(r, e, g, 0, L, True) for (r, e, g) in rjobs[:R - n_split]]
    halves = []
    for (r, e, g) in rjobs[R - n_split:]:
        halves.append((r, e, g, 0, L // 2, True))
        halves.append((r, e, g, L // 2, L // 2, True))
    while len(whole) < whole_cap:
        src = whole[-1] if whole else (halves[-1][0], halves[-1][1],
                                       halves[-1][2], 0, L, True)
        whole.append((*src[:5], False))
    while len(halves) < half_cap:
        halves.append((*halves[-1][:5], False))
    whole.sort(key=lambda t: (t[1], -t[2], t[0]))   # group by expert
    halves.sort(key=lambda t: (t[1], -t[2], t[0], t[3]))

    # gen jobs: one hybrid (fp8 mm1 + fp16 mm2) per core, rest full fp16.
    # Slot order [fp8 wholes, hybrid gen, fp8 halves, fp16 gens] staggers the
    # WAR-gated weight loads so each lands well before its consumer.
    nhyb = 1 if GPC >= 1 else 0
    slots_mode = ([(L, 8)] * nf + [(L, 9)] * nhyb + [(L // 2, 8)] * nh
                  + [(L, 16)] * (GPC - nhyb))
    assign = []
    for c in range(NCORES):
        row_jobs = [whole[c * nf + s] for s in range(nf)]
        row_jobs += [(GPC * c + i, GEN, 1.0, 0, L, True) for i in range(nhyb)]
        row_jobs += [halves[c * nh + s] for s in range(nh)]
        row_jobs += [(GPC * c + i, GEN, 1.0, 0, L, True)
                     for i in range(nhyb, GPC)]
        assign.append(row_jobs)

    # weight-load schedules per pool: adjacent dedupe, core-uniform
    users = {"w18": (8, 9), "w28": (8,), "w116": (16,), "w216": (9, 16)}
    prev = dict.fromkeys(users)
    cnt = dict.fromkeys(users, 0)
    slots = []
    for s, (tok, mode) in enumerate(slots_mode):
        sets = tuple(assign[c][s][1] for c in range(NCORES))
        lw = dict.fromkeys(users)
        for pool, modes in users.items():
            if mode in modes and sets != prev[pool]:
                lw[pool] = cnt[pool]
                cnt[pool] += 1
                prev[pool] = sets
        slots.append((tok, mode, lw["w18"], lw["w28"], lw["w116"], lw["w216"]))
    return ((cnt["w18"], cnt["w28"], cnt["w116"], cnt["w216"], fold_gb,
             tuple(slots)), assign)


def kernel(cycle_curve_data, cycle_numbers, DKP_embeddings,
           gate_We, gate_Wc, gate_b, gate_Wo, gate_bo,
           e_w1, e_b1, e_w2, e_b2, e_gamma, e_beta,
           g_w1, g_b1, g_w2, g_b2, g_gamma, g_beta):
    x = np.asarray(cycle_curve_data, dtype=np.float32)
    idx, gated = _router(np.asarray(cycle_numbers, np.float32),
                         np.asarray(DKP_embeddings, np.float32),
                         np.asarray(gate_We, np.float32),
                         np.asarray(gate_Wc, np.float32),
                         np.asarray(gate_b, np.float32),
                         np.asarray(gate_Wo, np.float32),
                         np.asarray(gate_bo, np.float32))

    w1s = {**{e: np.asarray(e_w1[e]) for e in range(E)}, GEN: np.asarray(g_w1)}
    w2s = {**{e: np.asarray(e_w2[e]) for e in range(E)}, GEN: np.asarray(g_w2)}
    b1s = {**{e: np.asarray(e_b1[e]) for e in range(E)}, GEN: np.asarray(g_b1)}
    b2s = {**{e: np.asarray(e_b2[e]) for e in range(E)}, GEN: np.asarray(g_b2)}
    gms = {**{e: np.asarray(e_gamma[e]) for e in range(E)}, GEN: np.asarray(g_gamma)}
    bts = {**{e: np.asarray(e_beta[e]) for e in range(E)}, GEN: np.asarray(g_beta)}

    # gamma/beta fold: when every expert's gamma is a constant vector and
    # beta is zero, the gate*gamma scale folds into the rsqrt constant and
    # the per-chunk gamma/beta DVE passes are skipped.
    fold_gb = all(
        np.all(gms[s] == gms[s].flat[0]) and not np.any(bts[s])
        for s in list(range(E)) + [GEN])
    spec, assign = _schedule(idx, gated, fold_gb)
    n18, n28, n116, n216, fold_gb, slots = spec
    nc = _build_nc(spec)

    # weight images, shared across cores where sets coincide
    w18_img, w28_img, w116_img, w216_img = {}, {}, {}, {}
    for c in range(NCORES):
        for j, (tok, mode, lw18, lw28, lw116, lw216) in enumerate(slots):
            s = assign[c][j][1]
            if mode in (8, 9) and s not in w18_img:
                w18_img[s] = (w1s[s] * W1_SCALE).astype(F8NP)
            if mode == 8 and s not in w28_img:
                w28_img[s] = (w2s[s] * W2_SCALE).astype(F8NP)
            if mode == 16 and s not in w116_img:
                w116_img[s] = w1s[s].astype(np.float16)
            if mode in (9, 16) and s not in w216_img:
                w216_img[s] = w2s[s].astype(np.float16)

    in_maps = []
    for c in range(NCORES):
        im = {}
        if n18:
            im["w18"] = w18_st = np.empty((n18, D, DFF), F8NP)
        if n28:
            im["w28"] = w28_st = np.empty((n28, DFF, D), F8NP)
        if n116:
            im["w116"] = w116_st = np.empty((n116, D, DFF), np.float16)
        if n216:
            im["w216"] = w216_st = np.empty((n216, DFF, D), np.float16)
        b1_st = np.empty((128, len(slots), MC1), np.float32)
        gb_st = np.empty((len(slots), 2, D), np.float16)
        lnc_st = np.empty((128, len(slots), 2), np.float32)
        for j, (tok, mode, lw18, lw28, lw116, lw216) in enumerate(slots):
            r, s, g, off, cnt, real = assign[c][j]
            if lw18 is not None:
                w18_st[lw18] = w18_img[s]
            if lw28 is not None:
                w28_st[lw28] = w28_img[s]
            if lw116 is not None:
                w116_st[lw116] = w116_img[s]
            if lw216 is not None:
                w216_st[lw216] = w216_img[s]
            xt = x[r].T[:, off:off + cnt]
            if mode in (8, 9):
                im[f"xt{j}"] = np.ascontiguousarray(xt).astype(F8NP)
            else:
                im[f"xt{j}"] = np.ascontiguousarray(xt).astype(np.float16)
            xr_scale = W2_SCALE if mode == 8 else 1.0
            im[f"xr{j}"] = ((x[r, off:off + cnt] + b2s[s]) *
                            xr_scale).astype(np.float16)
            b1_st[:, j, :] = b1s[s].reshape(MC1, 128).T
            gb_st[j, 0] = g * gms[s]
            gb_st[j, 1] = g * bts[s]
            cg = float(g * gms[s].flat[0]) if fold_gb else 1.0
            lnc_st[:, j, 0] = 1.0 / (cg * cg)
            lnc_st[:, j, 1] = LN_EPS / (cg * cg)
        im["b1"], im["gb"], im["lnc"] = b1_st, gb_st, lnc_st
        in_maps.append(im)

    res = bass_utils.run_bass_kernel_spmd(nc, in_maps, core_ids=list(range(NCORES)))
    global last_run
    last_run = res

    # Combine: out[r] = y_general(r) + bf16(sum of gated expert outputs).
    import ml_dtypes
    gen_out = np.zeros((B, L, D), np.float32)
    comb = np.zeros((B, L, D), np.float32)
    for c in range(NCORES):
        y = res.results[c]
        for j, (tok, mode, lw18, lw28, lw116, lw216) in enumerate(slots):
            r, s, g, off, cnt, real = assign[c][j]
            if not real:
                continue
            if s == GEN:
                gen_out[r, off:off + cnt] = y[f"y{j}"].astype(np.float32)
            else:
                comb[r, off:off + cnt] += y[f"y{j}"].astype(np.float32)
    return gen_out + comb.astype(ml_dtypes.bfloat16).astype(np.float32)


# revision 25
# speedup vs baseline: 1.3006x; 1.3006x over previous
"""Trainium2 Bass kernel for nn_IntraCycleMoELayer (MoE routing, 8 cores).

Strategy
--------
The reference computes all E=8 experts densely, but the top-2 gate zeroes all
but 2 experts per batch row.  Real work per row: 2 routed expert MLP blocks +
1 general MLP block of
  LN(gelu_tanh(x @ w1 + b1) @ w2 + b2 + x) * gamma + beta
over [L=512 tokens, D=768] with DFF=3072.  Routed jobs with a renormalized
gate below GATE_SKIP (1e-2) are dropped (~0.1% output error).

Precision plan (gate: rel_err < 2e-2; fp8-e4m3 DoubleRow gives 2x PE
throughput): routed experts run fully in fp8; the general expert (weight 1.0
in the output sum, the accuracy-critical term) runs with fp8 mm1 + fp16 mm2
on half the rows and fully fp16 on the rest.  Remaining routed work is
token-split so all 8 SPMD cores carry the same slot structure:
  [fp8 whole, fp8 whole, hybrid gen, fp8 half, fp16 gen]
fp8 accuracy tricks:
  * w1 is pre-scaled by 128 and w2 by 256 before e4m3 quantization so the
    weight magnitudes (std ~0.02-0.04) land in e4m3's normal range instead
    of its denormals (this alone is worth ~2x in error).  The w1 scale is
    undone for free by the gelu activation's `scale` operand; the w2 scale
    is never undone - the residual xr is pre-scaled by 256 on the host and
    LayerNorm is scale invariant.
  * gelu output h is quantized to fp8 (routed) / fp16 (hybrid) directly by
    the scalar engine.
  * gate*gamma folds into the LN rstd constant (c_mul staged per slot), so
    no per-chunk gamma/beta DVE passes are needed when gamma is constant
    and beta is zero; rstd = rsqrt(var*c_mul + c_add) is computed on the
    DVE with the int32 exponent-halving seed + one Newton step (no
    scalar-engine Sqrt => no activation-table thrash against gelu).

Engine/ring discipline (descriptor ISSUE costs ~650ns on the issuing
engine, and a waiting descriptor head-blocks its ring):
  * sync ring: slot-0 critical xt + w1 first, deferred non-critical early
    loads behind an early mm1 matmul, then y outputs (fp16).
  * gpsimd ring: only never-blocking early bulk (b1/lnc, first w2, xr).
  * every later WAR-gated weight/xr load issues from the scalar engine's
    instruction stream at its slot, where program order has already
    released the target buffer - so no ring ever stalls on a WAR wait.
"""
import numpy as np

import concourse.bass as bass
import concourse.mybir as mybir
import concourse.tile as tile
from concourse import bacc
from concourse.bass import ts
from concourse import bass_utils

B, L, D, DFF, DLLM, E, TOPK = 16, 512, 768, 3072, 4096, 8, 2
EPS_GATE = 1e-9
LN_EPS = 1e-5
NCORES = 8
GEN = E                               # weight-set id of the general expert
GATE_SKIP = 0.01                      # drop routed jobs with tiny gates
W1_SCALE = 128.0                      # fp8 weight pre-scales (powers of 2)
W2_SCALE = 256.0
KC1, MC1 = D // 128, DFF // 128       # 6, 24
KC2 = DFF // 128                      # 24
GPC = B // NCORES                     # general jobs per core = 2
dt = mybir.dt
F8NP = dt.np(dt.float8e4)
# the MLP nonlinearity; CoreSim lacks Gelu_apprx_tanh, so sim tests swap this
_ACT_FUNC = mybir.ActivationFunctionType.Gelu_apprx_tanh

_cache = {}  # spec -> finalized nc


def _router(cycle_numbers, DKP_embeddings, gate_We, gate_Wc, gate_b, gate_Wo,
            gate_bo):
    """Replicate the reference router in fp32 numpy: top-2 indices + gates."""
    h = np.maximum(
        DKP_embeddings @ gate_We + cycle_numbers @ gate_Wc + gate_b, 0.0)
    logits = h @ gate_Wo + gate_bo                       # [B, E]
    idx = np.argsort(-logits, axis=1, kind="stable")[:, :TOPK]
    m = logits.max(axis=1, keepdims=True)
    p = np.exp(logits - m)
    p /= p.sum(axis=1, keepdims=True)
    mask = np.zeros_like(p)
    mask[np.arange(logits.shape[0])[:, None], idx] = 1.0
    gated = p * mask
    gated = gated / (gated.sum(axis=1, keepdims=True) + EPS_GATE)
    return idx, gated


def _build_nc(spec):
    """Build the SPMD per-core program.

    spec = (n18, n28, n116, n216, fold_gb, slots); each slot is
      (tok, mode, lw18, lw28, lw116, lw216):
      tok  - token count for this slot (multiple of 128)
      mode - 8 (fp8 DoubleRow), 9 (hybrid: fp8 mm1 + fp16 mm2), 16 (fp16)
      lw** - weight-pool slot to DMA before this job (None = reuse previous)
    """
    if spec in _cache:
        return _cache[spec]
    n18, n28, n116, n216, fold_gb, slots = spec
    n_slots = len(slots)

    nc = bacc.Bacc("TRN2", target_bir_lowering=False, debug=False)
    w18_d = w28_d = w116_d = w216_d = None
    if n18:
        w18_d = nc.dram_tensor("w18", [n18, D, DFF], dt.float8e4, kind="ExternalInput")
    if n28:
        w28_d = nc.dram_tensor("w28", [n28, DFF, D], dt.float8e4, kind="ExternalInput")
    if n116:
        w116_d = nc.dram_tensor("w116", [n116, D, DFF], dt.float16, kind="ExternalInput")
    if n216:
        w216_d = nc.dram_tensor("w216", [n216, DFF, D], dt.float16, kind="ExternalInput")
    xt_d, xr_d, y_d = [], [], []
    for j, (tok, mode, _, _, _, _) in enumerate(slots):
        xdt = dt.float8e4 if mode in (8, 9) else dt.float16
        xt_d.append(nc.dram_tensor(f"xt{j}", [D, tok], xdt, kind="ExternalInput"))
        xr_d.append(nc.dram_tensor(f"xr{j}", [tok, D], dt.float16, kind="ExternalInput"))
        y_d.append(nc.dram_tensor(f"y{j}", [tok, D], dt.float16, kind="ExternalOutput"))
    b1_d = nc.dram_tensor("b1", [128, n_slots, MC1], dt.float32, kind="ExternalInput")
    gb_d = nc.dram_tensor("gb", [n_slots, 2, D], dt.float16, kind="ExternalInput")
    # per-slot LN constants: [:,:,0]=c_mul, [:,:,1]=c_add so that
    # rstd' = rsqrt(var*c_mul + c_add); with fold_gb, c_mul=1/(g*gamma)^2
    # and the gamma/beta DVE passes are skipped entirely.
    lnc_d = nc.dram_tensor("lnc", [128, n_slots, 2], dt.float32, kind="ExternalInput")

    gelu = _ACT_FUNC
    DR = mybir.MatmulPerfMode.DoubleRow
    nf8slots = sum(1 for s in slots if s[1] in (8, 9))

    with tile.TileContext(nc) as tc, \
         tc.tile_pool(name="w18p", bufs=1) as w18p, \
         tc.tile_pool(name="w28p", bufs=1) as w28p, \
         tc.tile_pool(name="w116p", bufs=1) as w116p, \
         tc.tile_pool(name="w216p", bufs=1) as w216p, \
         tc.tile_pool(name="xt8p", bufs=min(3, max(1, nf8slots))) as xt8p, \
         tc.tile_pool(name="xt16p", bufs=1) as xt16p, \
         tc.tile_pool(name="h8p", bufs=2) as h8p, \
         tc.tile_pool(name="h16p", bufs=1) as h16p, \
         tc.tile_pool(name="xrp", bufs=2) as xrp, \
         tc.tile_pool(name="gbp", bufs=2) as gbp, \
         tc.tile_pool(name="rp", bufs=3) as rp, \
         tc.tile_pool(name="sp", bufs=2) as sp, \
         tc.tile_pool(name="cp", bufs=1) as cp, \
         tc.tile_pool(name="php", bufs=4, space="PSUM") as php, \
         tc.tile_pool(name="pop", bufs=2, space="PSUM") as pop:

        from concourse.bass import _add_dep_helper

        b1_all = cp.tile([128, n_slots, MC1], dt.float32)
        lnc_all = cp.tile([128, n_slots, 2], dt.float32)

        # PE warmup: matmuls on zeros so the HAM clock-gate ramps while the
        # first weight DMAs are in flight (sized to end as slot0's w1 lands).
        warm_z = cp.tile([128, 512], dt.float16)
        nc.vector.memset(warm_z, 0.0)
        for _ in range(10):
            wp_t = php.tile([128, 512], dt.float32, tag="ph")
            nc.tensor.matmul(wp_t, lhsT=warm_z[:, 0:128], rhs=warm_z,
                             start=True, stop=True)

        first_mm = None      # anchor for delaying non-critical head DMAs
        deferred = []        # DMA insts to hook behind first_mm

        def dma(engine, dst, src, defer):
            d = engine.dma_start(dst, src)
            if defer:
                deferred.append(d)
            return d

        # ---- weight / xt loads, emitted with explicit queue ordering ----
        w18_sb = [None] * n_slots
        w28_sb = [None] * n_slots
        w116_sb = [None] * n_slots
        w216_sb = [None] * n_slots
        xt_sb = [None] * n_slots
        H = DFF // 2

        # per-k(-group) full-row descriptors: DMA descriptor ISSUE costs
        # ~650ns on the issuing engine, so fewer/bigger beats many small.
        def load_w18(j, u, eng, defer):
            t = w18p.tile([128, KC1, DFF], dt.float8e4, tag="w18")
            src = w18_d[u].rearrange("(ko p) n -> p ko n", p=128)
            for k in range(KC1):
                dma(eng, t[:, k, :], src[:, k, :], defer)
            w18_sb[j] = t

        def load_w28(j, u, eng, defer):
            t = w28p.tile([128, KC2, D], dt.float8e4, tag="w28")
            src = w28_d[u].rearrange("(ko p) n -> p ko n", p=128)
            for k in range(0, KC2, 6):
                dma(eng, t[:, k:k + 6, :], src[:, k:k + 6, :], defer)
            w28_sb[j] = t

        def load_w116(j, u, eng, defer):
            t = w116p.tile([128, KC1, DFF], dt.float16, tag="w116")
            src = w116_d[u].rearrange("(ko p) n -> p ko n", p=128)
            for k in range(0, KC1, 2):
                dma(eng, t[:, k:k + 2, :], src[:, k:k + 2, :], defer)
            w116_sb[j] = t

        def load_w216(j, u, eng, defer):
            t = w216p.tile([128, KC2, D], dt.float16, tag="w216")
            src = w216_d[u].rearrange("(ko p) n -> p ko n", p=128)
            for k in range(0, KC2, 6):
                dma(eng, t[:, k:k + 6, :], src[:, k:k + 6, :], defer)
            w216_sb[j] = t

        def load_xt(j, defer):
            tok, mode = slots[j][0], slots[j][1]
            if mode in (8, 9):
                t = xt8p.tile([128, KC1, tok], dt.float8e4, tag="xt8")
            else:
                t = xt16p.tile([128, KC1, tok], dt.float16, tag="xt16")
            src = xt_d[j].rearrange("(ko p) t -> p ko t", p=128)
            dma(nc.sync, t, src, defer)
            xt_sb[j] = t

        xr_sb = [None] * n_slots
        gb_sb = [None] * n_slots

        def load_xr_gb(j, eng, defer):
            tok = slots[j][0]
            t = xrp.tile([128, tok // 128, D], dt.float16, tag="xr")
            src = xr_d[j].rearrange("(t p) d -> p t d", p=128)
            dma(eng, t, src, defer)
            xr_sb[j] = t
            if not fold_gb:   # gamma/beta tiles only needed un-folded
                g = gbp.tile([128, 2, D], dt.float16, tag="gb")
                gap = gb_d[j]
                dma(eng, g,
                    bass.AP(tensor=gap.tensor, offset=gap.offset,
                            ap=[[0, 128], *gap.ap]), defer)
                gb_sb[j] = g

        # Ring discipline:
        #   sync:   xt0, w18[0] | deferred: other xt, first w116 | y outs
        #   gpsimd: b1/lnc | deferred: first w28, xr's, first w216
        #   scalar stream (at the owning slot, in program order): every
        #     later (WAR-gated) weight load - its program position already
        #     satisfies the WAR, so no ring is ever head-blocked by a wait.
        f8_js = [j for j in range(n_slots) if slots[j][1] in (8, 9)]
        gen_js = [j for j in range(n_slots) if slots[j][1] == 16]
        j0 = 0
        load_xt(j0, defer=False)
        tok0, mode0, w18l0, w28l0, w116l0, w216l0 = slots[j0]
        if w18l0 is not None:
            load_w18(j0, w18l0, nc.sync, defer=False)
        if w116l0 is not None:
            load_w116(j0, w116l0, nc.sync, defer=False)
        dma(nc.gpsimd, b1_all, b1_d[:], False)
        dma(nc.gpsimd, lnc_all, lnc_d[:], False)
        for j in range(1, n_slots):
            load_xt(j, defer=True)
        if w28l0 is not None:
            load_w28(j0, w28l0, nc.gpsimd, defer=True)
        if w216l0 is not None:
            load_w216(j0, w216l0, nc.gpsimd, defer=True)
        for j in f8_js[:2]:   # first two fit xrp's bufs without WAR waits
            load_xr_gb(j, nc.gpsimd, defer=True)
        # first-use (non-WAR) pool loads of later slots: fine on the rings
        done_first = {"w116": w116l0 is not None, "w216": w216l0 is not None}
        for j in range(1, n_slots):
            if slots[j][4] is not None and not done_first["w116"]:
                load_w116(j, slots[j][4], nc.sync, defer=True)
                done_first["w116"] = True
            if slots[j][5] is not None and not done_first["w216"]:
                load_w216(j, slots[j][5], nc.gpsimd, defer=True)
                done_first["w216"] = True

        # ---- compute ----
        last = {"w18": w18_sb[j0], "w28": w28_sb[j0],
                "w116": w116_sb[j0], "w216": w216_sb[j0]}
        for j, (tok, mode, lw18, lw28, lw116, lw216) in enumerate(slots):
            TCj = tok // 128
            if xr_sb[j] is None:
                # WAR-gated xr loads issue from the scalar stream here;
                # program order has already released the buffer.
                load_xr_gb(j, nc.scalar, defer=False)
            # later (WAR-gated) weight loads: issue from the scalar stream
            # right here; program order has already released the buffers.
            if j != j0:
                if lw18 is not None and w18_sb[j] is None:
                    load_w18(j, lw18, nc.scalar, defer=False)
                if lw28 is not None and w28_sb[j] is None:
                    load_w28(j, lw28, nc.scalar, defer=False)
                if lw116 is not None and w116_sb[j] is None:
                    load_w116(j, lw116, nc.scalar, defer=False)
                if lw216 is not None and w216_sb[j] is None:
                    load_w216(j, lw216, nc.scalar, defer=False)
            for pool, sb in (("w18", w18_sb), ("w28", w28_sb),
                             ("w116", w116_sb), ("w216", w216_sb)):
                if sb[j] is not None:
                    last[pool] = sb[j]
                else:
                    sb[j] = last[pool]

            # mm1 + gelu: h^T [DFF on partitions, tokens free]
            if mode == 8:
                h_sb = h8p.tile([128, KC2, tok], dt.float8e4, tag="h8")
            else:
                h_sb = h16p.tile([128, KC2, tok], dt.float16, tag="h16")
            for m in range(MC1):
                ph = php.tile([128, 512], dt.float32, tag="ph")
                if mode in (8, 9):
                    for kp in range(KC1 // 2):
                        mm = nc.tensor.matmul(
                            ph[:, 0:tok],
                            lhsT=w18_sb[j][:, 2 * kp:2 * kp + 2, ts(m, 128)],
                            rhs=xt_sb[j][:, 2 * kp:2 * kp + 2, :],
                            start=(kp == 0), stop=(kp == KC1 // 2 - 1),
                            perf_mode=DR)
                        if first_mm is None and j == 0 and m == 12 and kp == 0:
                            first_mm = mm
                            for d in deferred:
                                _add_dep_helper(
                                    d.ins, first_mm.ins, sync=True,
                                    reason="delay non-critical head DMA")
                else:
                    for k in range(KC1):
                        mm = nc.tensor.matmul(
                            ph[:, 0:tok],
                            lhsT=w116_sb[j][:, k, ts(m, 128)],
                            rhs=xt_sb[j][:, k, :],
                            start=(k == 0), stop=(k == KC1 - 1))
                        if first_mm is None and j == 0 and m == 12 and k == 0:
                            first_mm = mm
                            for d in deferred:
                                _add_dep_helper(
                                    d.ins, first_mm.ins, sync=True,
                                    reason="delay non-critical head DMA")
                nc.scalar.activation(
                    out=h_sb[:, m, :], in_=ph[:, 0:tok], func=gelu,
                    bias=b1_all[:, j, m:m + 1],
                    scale=(1.0 / W1_SCALE) if mode in (8, 9) else 1.0)

            # mm2 + residual + LN per 128-token chunk; rstd comes from a
            # DVE-only Newton rsqrt (no scalar-engine act-table involvement),
            # so each chunk completes independently right behind its matmuls.
            for t in range(TCj):
                po = pop.tile([128, D], dt.float32, tag="po")
                if mode == 8:
                    for kp in range(KC2 // 2):
                        lh = h_sb[:, 2 * kp:2 * kp + 2, ts(t, 128)]
                        nc.tensor.matmul(po[:, 0:512], lhsT=lh,
                                         rhs=w28_sb[j][:, 2 * kp:2 * kp + 2, 0:512],
                                         start=(kp == 0), stop=(kp == KC2 // 2 - 1),
                                         perf_mode=DR)
                        nc.tensor.matmul(po[:, 512:D], lhsT=lh,
                                         rhs=w28_sb[j][:, 2 * kp:2 * kp + 2, 512:D],
                                         start=(kp == 0), stop=(kp == KC2 // 2 - 1),
                                         perf_mode=DR)
                else:
                    for k in range(KC2):
                        lh = h_sb[:, k, ts(t, 128)]
                        nc.tensor.matmul(po[:, 0:512], lhsT=lh,
                                         rhs=w216_sb[j][:, k, 0:512],
                                         start=(k == 0), stop=(k == KC2 - 1))
                        nc.tensor.matmul(po[:, 512:D], lhsT=lh,
                                         rhs=w216_sb[j][:, k, 512:D],
                                         start=(k == 0), stop=(k == KC2 - 1))
                r_sb = rp.tile([128, D], dt.float32, tag="r")
                nc.vector.tensor_add(r_sb, po, xr_sb[j][:, t, :])
                stats = sp.tile([128, 3, 6], dt.float32, tag="st")
                for s in range(3):
                    nc.vector.bn_stats(stats[:, s, :], r_sb[:, ts(s, 256)])
                mv = sp.tile([128, 2], dt.float32, tag="mv")
                nc.vector.bn_aggr(mv, stats)
                # rstd' = rsqrt(var*c_mul + c_add), one Newton iteration
                # seeded by the int32 exponent-halving trick, on the DVE.
                v = sp.tile([128, 1], dt.float32, tag="v")
                nc.vector.tensor_scalar(out=v, in0=mv[:, 1:2],
                                        scalar1=lnc_all[:, j, 0:1],
                                        scalar2=lnc_all[:, j, 1:2],
                                        op0=mybir.AluOpType.mult,
                                        op1=mybir.AluOpType.add)
                yb = sp.tile([128, 1], dt.int32, tag="yb")
                nc.vector.tensor_scalar(out=yb, in0=v.bitcast(dt.int32),
                                        scalar1=1, scalar2=None,
                                        op0=mybir.AluOpType.logical_shift_right)
                nc.vector.tensor_scalar(out=yb, in0=yb,
                                        scalar1=-1, scalar2=0x5f3759df,
                                        op0=mybir.AluOpType.mult,
                                        op1=mybir.AluOpType.add)
                yf = yb.bitcast(dt.float32)
                nt = sp.tile([128, 1], dt.float32, tag="nt")
                nc.vector.tensor_mul(nt, yf, yf)
                nc.vector.tensor_mul(nt, nt, v)
                nc.gpsimd.tensor_scalar(out=nt, in0=nt, scalar1=-0.5,
                                        scalar2=1.5,
                                        op0=mybir.AluOpType.mult,
                                        op1=mybir.AluOpType.add)
                nc.vector.tensor_mul(yf, yf, nt)
                r16 = rp.tile([128, D], dt.float16, tag="r16")
                nc.vector.tensor_scalar(out=(r16 if fold_gb else r_sb),
                                        in0=r_sb,
                                        scalar1=mv[:, 0:1],
                                        scalar2=yf,
                                        op0=mybir.AluOpType.subtract,
                                        op1=mybir.AluOpType.mult)
                if not fold_gb:
                    nc.vector.tensor_mul(r_sb, r_sb, gb_sb[j][:, 0, :])
                    nc.vector.tensor_add(r16, r_sb, gb_sb[j][:, 1, :])
                nc.sync.dma_start(
                    y_d[j].rearrange("(t p) d -> p t d", p=128)[:, t, :], r16)

    nc.finalize()
    _cache[spec] = nc
    return nc


def _schedule(idx, gated, fold_gb):
    """Build the SPMD slot structure + per-core assignment.

    Returns (spec, assign) where assign[c][j] =
      (row, set_id, gate, tok_off, tok_cnt, real)
    """
    rjobs = []
    for r in range(B):
        for e in idx[r]:
            g = float(gated[r, e])
            if g >= GATE_SKIP:
                rjobs.append((r, int(e), g))
    # keep the largest-gate jobs whole; split the smallest-gate ones
    rjobs.sort(key=lambda t: -t[2])
    R = len(rjobs)
    hpc = -(-2 * R // NCORES)            # routed half-slots per core
    nf, nh = hpc // 2, hpc % 2
    whole_cap, half_cap = NCORES * nf, NCORES * nh
    n_split = max(0, R - whole_cap)
    whole = [(r, e, g, 0, L, True) for (r, e, g) in rjobs[:R - n_split]]
    halves = []
    for (r, e, g) in rjobs[R - n_split:]:
        halves.append((r, e, g, 0, L // 2, True))
        halves.append((r, e, g, L // 2, L // 2, True))
    while len(whole) < whole_cap:
        src = whole[-1] if whole else (halves[-1][0], halves[-1][1],
                                       halves[-1][2], 0, L, True)
        whole.append((*src[:5], False))
    while len(halves) < half_cap:
        halves.append((*halves[-1][:5], False))
    whole.sort(key=lambda t: (t[1], -t[2], t[0]))   # group by expert
    halves.sort(key=lambda t: (t[1], -t[2], t[0], t[3]))

    # gen jobs: one hybrid (fp8 mm1 + fp16 mm2) per core, rest full fp16.
    # Slot order [fp8 wholes, hybrid gen, fp8 halves, fp16 gens] staggers the
    # WAR-gated weight loads so each lands well before its consumer.
    nhyb = 1 if GPC >= 1 else 0
    slots_mode = ([(L, 8)] * nf + [(L, 9)] * nhyb + [(L // 2, 8)] * nh
                  + [(L, 16)] * (GPC - nhyb))
    assign = []
    for c in range(NCORES):
        row_jobs = [whole[c * nf + s] for s in range(nf)]
        row_jobs += [(GPC * c + i, GEN, 1.0, 0, L, True) for i in range(nhyb)]
        row_jobs += [halves[c * nh + s] for s in range(nh)]
        row_jobs += [(GPC * c + i, GEN, 1.0, 0, L, True)
                     for i in range(nhyb, GPC)]
        assign.append(row_jobs)

    # weight-load schedules per pool: adjacent dedupe, core-uniform
    users = {"w18": (8, 9), "w28": (8,), "w116": (16,), "w216": (9, 16)}
    prev = dict.fromkeys(users)
    cnt = dict.fromkeys(users, 0)
    slots = []
    for s, (tok, mode) in enumerate(slots_mode):
        sets = tuple(assign[c][s][1] for c in range(NCORES))
        lw = dict.fromkeys(users)
        for pool, modes in users.items():
            if mode in modes and sets != prev[pool]:
                lw[pool] = cnt[pool]
                cnt[pool] += 1
                prev[pool] = sets
        slots.append((tok, mode, lw["w18"], lw["w28"], lw["w116"], lw["w216"]))
    return ((cnt["w18"], cnt["w28"], cnt["w116"], cnt["w216"], fold_gb,
             tuple(slots)), assign)


def kernel(cycle_curve_data, cycle_numbers, DKP_embeddings,
           gate_We, gate_Wc, gate_b, gate_Wo, gate_bo,
           e_w1, e_b1, e_w2, e_b2, e_gamma, e_beta,
           g_w1, g_b1, g_w2, g_b2, g_gamma, g_beta):
    x = np.asarray(cycle_curve_data, dtype=np.float32)
    idx, gated = _router(np.asarray(cycle_numbers, np.float32),
                         np.asarray(DKP_embeddings, np.float32),
                         np.asarray(gate_We, np.float32),
                         np.asarray(gate_Wc, np.float32),
                         np.asarray(gate_b, np.float32),
                         np.asarray(gate_Wo, np.float32),
                         np.asarray(gate_bo, np.float32))

    w1s = {**{e: np.asarray(e_w1[e]) for e in range(E)}, GEN: np.asarray(g_w1)}
    w2s = {**{e: np.asarray(e_w2[e]) for e in range(E)}, GEN: np.asarray(g_w2)}
    b1s = {**{e: np.asarray(e_b1[e]) for e in range(E)}, GEN: np.asarray(g_b1)}
    b2s = {**{e: np.asarray(e_b2[e]) for e in range(E)}, GEN: np.asarray(g_b2)}
    gms = {**{e: np.asarray(e_gamma[e]) for e in range(E)}, GEN: np.asarray(g_gamma)}
    bts = {**{e: np.asarray(e_beta[e]) for e in range(E)}, GEN: np.asarray(g_beta)}

    # gamma/beta fold: when every expert's gamma is a constant vector and
    # beta is zero, the gate*gamma scale folds into the rsqrt constant and
    # the per-chunk gamma/beta DVE passes are skipped.
    fold_gb = all(
        np.all(gms[s] == gms[s].flat[0]) and not np.any(bts[s])
        for s in list(range(E)) + [GEN])
    spec, assign = _schedule(idx, gated, fold_gb)
    n18, n28, n116, n216, fold_gb, slots = spec
    nc = _build_nc(spec)

    # weight images, shared across cores where sets coincide
    w18_img, w28_img, w116_img, w216_img = {}, {}, {}, {}
    for c in range(NCORES):
        for j, (tok, mode, lw18, lw28, lw116, lw216) in enumerate(slots):
            s = assign[c][j][1]
            if mode in (8, 9) and s not in w18_img:
                w18_img[s] = (w1s[s] * W1_SCALE).astype(F8NP)
            if mode == 8 and s not in w28_img:
                w28_img[s] = (w2s[s] * W2_SCALE).astype(F8NP)
            if mode == 16 and s not in w116_img:
                w116_img[s] = w1s[s].astype(np.float16)
            if mode in (9, 16) and s not in w216_img:
                w216_img[s] = w2s[s].astype(np.float16)

    in_maps = []
    for c in range(NCORES):
        im = {}
        if n18:
            im["w18"] = w18_st = np.empty((n18, D, DFF), F8NP)
        if n28:
            im["w28"] = w28_st = np.empty((n28, DFF, D), F8NP)
        if n116:
            im["w116"] = w116_st = np.empty((n116, D, DFF), np.float16)
        if n216:
            im["w216"] = w216_st = np.empty((n216, DFF, D), np.float16)
        b1_st = np.empty((128, len(slots), MC1), np.float32)
        gb_st = np.empty((len(slots), 2, D), np.float16)
        lnc_st = np.empty((128, len(slots), 2), np.float32)
        for j, (tok, mode, lw18, lw28, lw116, lw216) in enumerate(slots):
            r, s, g, off, cnt, real = assign[c][j]
            if lw18 is not None:
                w18_st[lw18] = w18_img[s]
            if lw28 is not None:
                w28_st[lw28] = w28_img[s]
            if lw116 is not None:
                w116_st[lw116] = w116_img[s]
            if lw216 is not None:
                w216_st[lw216] = w216_img[s]
            xt = x[r].T[:, off:off + cnt]
            if mode in (8, 9):
                im[f"xt{j}"] = np.ascontiguousarray(xt).astype(F8NP)
            else:
                im[f"xt{j}"] = np.ascontiguousarray(xt).astype(np.float16)
            xr_scale = W2_SCALE if mode == 8 else 1.0
            im[f"xr{j}"] = ((x[r, off:off + cnt] + b2s[s]) *
                            xr_scale).astype(np.float16)
            b1_st[:, j, :] = b1s[s].reshape(MC1, 128).T
            gb_st[j, 0] = g * gms[s]
            gb_st[j, 1] = g * bts[s]
            cg = float(g * gms[s].flat[0]) if fold_gb else 1.0
            lnc_st[:, j, 0] = 1.0 / (cg * cg)
            lnc_st[:, j, 1] = LN_EPS / (cg * cg)
        im["b1"], im["gb"], im["lnc"] = b1_st, gb_st, lnc_st
        in_maps.append(im)

    res = bass_utils.run_bass_kernel_spmd(nc, in_maps, core_ids=list(range(NCORES)))
    global last_run
    last_run = res

    # Combine: out[r] = y_general(r) + bf16(sum of gated expert outputs).
    import ml_dtypes
    gen_out = np.zeros((B, L, D), np.float32)
    comb = np.zeros((B, L, D), np.float32)
    for c in range(NCORES):
        y = res.results[c]
        for j, (tok, mode, lw18, lw28, lw116, lw216) in enumerate(slots):
            r, s, g, off, cnt, real = assign[c][j]
            if not real:
                continue
            if s == GEN:
                gen_out[r, off:off + cnt] = y[f"y{j}"].astype(np.float32)
            else:
                comb[r, off:off + cnt] += y[f"y{j}"].astype(np.float32)
    return gen_out + comb.astype(ml_dtypes.bfloat16).astype(np.float32)
